# revision 39
# baseline (speedup 1.0000x reference)
"""Distributed Trainium2 Bass kernel for GQA attention (B=2, S=2048, H=2048,
NH=16, NKV=4, HD=128) across 8 NeuronCores.

Sharding: core c -> (batch b = c//4, kv-group g = c%4).  Each core computes
Q/K/V projections for its 4 query heads + 1 kv head (column-sharded Wq/Wkv),
RoPE, causal flash-style attention in transposed layout (S^T = K Q^T so the
PV contraction lands on partitions), then a ROW-SHARDED Wo partial product:
y_partial^T[n, s] = Wo[own 512 rows, n]^T @ O_own^T[m, s].  Each core writes
its full-width [2048, 2048] bf16 partial y^T to DRAM; the host sums the 4
partials per batch during unshard (no on-device collective at all -- v2's
AllGather cost a 22us in-order PE stall mid-kernel plus a ~31us serialized
collective tail).

All matmul operands are bf16 (1 cycle/row on PE); accumulation is f32 in PSUM;
softmax runs without max-subtraction (scores are ~N(0,1), exp is safe in f32).
Causal structure is exploited at column granularity: for a diagonal k-chunk at
offset d, only score columns >= d are computed/exp'd/accumulated, and the
staircase boundary is handled by one [128,128] triangle multiply.

v3 schedule notes:
- Matmuls that share the same moving operand issue ~50ns faster back-to-back
  (213ns vs 262ns for 512-wide, measured on hw), so chains are interleaved in
  pairs everywhere: Q projection head-pairs share the moving xt chunk, V's
  four s-chunks share the moving wkv chunk (ho-outer loop), score chunk pairs
  share the moving q, and Wo n-chunk pairs share the moving O^T chunk.
- PSUM pools: proj 2 + scores/rope/denom 2 + PV out 2 + Wo 2 = 8 banks.
- Wo partials are evacuated alternately on DVE and ACT so neither engine
  becomes the secondary bottleneck; output DMA rides the idle gpsimd queue.
"""

import math
import sys

sys.path.insert(0, "/opt/trn_rl_repo")

import numpy as np
import ml_dtypes

import concourse.bass as bass
import concourse.mybir as mybir
import concourse.tile as tile
from concourse import bacc
from concourse import bass_utils
from concourse.bass import ds, ts

BF16 = mybir.dt.bfloat16
F32 = mybir.dt.float32
AF = mybir.ActivationFunctionType

HD = 128      # head dim
GQ = 4        # query heads per core
QC = GQ * HD  # query columns per core (512)
SB = 512      # sequence block
P = 128


def build_kernel(S=2048, H=2048):
    NB = S // SB          # number of seq blocks
    HO = H // P           # hidden contraction chunks
    ST = SB // P          # seq tiles per block (4)
    NC = H // P           # output n-chunks (16) -- full width, row-sharded Wo

    nc = bacc.Bacc("TRN2", target_bir_lowering=False, debug=False, num_devices=8)

    # Host layouts give long per-partition DMA lines: xt [ho, p, s] (4KB
    # lines, one DMA per ho covers all 4 seq blocks), wq [p, ho, m] (loaded
    # in 4-ho groups, 4KB lines), wkv [p, {k,v}, ho, m] (one 8KB-line DMA),
    # cossin [j, p, kind, s] (2KB lines, one DMA per block), wo [p, m, n, c]
    # (one 4KB-line DMA per m); consts3 packs rotm|ident|trineg.
    xt = nc.dram_tensor("xt", [HO * P, S], BF16, kind="ExternalInput").ap()
    wq = nc.dram_tensor("wq", [P, 2 * HO * (QC // 2)], BF16, kind="ExternalInput").ap()
    wkv = nc.dram_tensor("wkv", [P, 2 * HO * HD], BF16, kind="ExternalInput").ap()
    # row-sharded Wo repacked as [p, m(4), n(16), 128]
    wo = nc.dram_tensor("wo", [P, 4 * NC * P], BF16, kind="ExternalInput").ap()
    cossin = nc.dram_tensor("cossin", [NB * P, 2 * SB], BF16, kind="ExternalInput").ap()
    consts3 = nc.dram_tensor("consts3", [P, 3 * HD], BF16, kind="ExternalInput").ap()
    # partial y^T [n, s] in bf16; host sums the 4 kv-group partials per batch
    out = nc.dram_tensor("out", [NC * NB * P, SB], BF16, kind="ExternalOutput").ap()

    xt_r = xt.rearrange("(ho p) s -> p ho s", p=P)
    wq_r = wq.rearrange("p (h ho m) -> p h ho m", h=2, m=QC // 2)
    wkv_r = wkv.rearrange("p (ho k m) -> p ho k m", k=2, m=HD)
    wo_r = wo.rearrange("p (m n c) -> p m n c", m=4, n=NC)
    cs_r = cossin.rearrange("(j p) (k s) -> p j k s", p=P, k=2)
    out_r = out.rearrange("(c j p) s -> p c j s", p=P, j=NB)  # c = n-chunk

    with tile.TileContext(nc) as tc:
        with (
            tc.tile_pool(name="consts", bufs=1) as consts,
            tc.tile_pool(name="wpool", bufs=1) as wpool,
            tc.tile_pool(name="xtp", bufs=1) as xtp,
            tc.tile_pool(name="kvp", bufs=1) as kvp,
            tc.tile_pool(name="qfp", bufs=3) as qfp,
            tc.tile_pool(name="ofp", bufs=4) as ofp,
            tc.tile_pool(name="work", bufs=3) as work,
            tc.tile_pool(name="ptp", bufs=3) as ptp,
            tc.tile_pool(name="yevac", bufs=4) as yevac,
            tc.tile_pool(name="psP", bufs=2, space="PSUM") as psP,
            tc.tile_pool(name="psS", bufs=2, space="PSUM") as psS,
            tc.tile_pool(name="psO", bufs=2, space="PSUM") as psO,
            tc.tile_pool(name="psW", bufs=2, space="PSUM") as psW,
        ):
            # ---- upfront loads: xt(0) and wq interleaved first (the Q
            # projection consumes them chunk-by-chunk as they land), then the
            # first rope tables, then everything else.  DMAs alternate between
            # the SP and Activation HWDGE queues for early bandwidth; loads
            # emitted after attention starts ride sync only (scalar queue is
            # the exp stream).
            # round-robin preamble loads over three queues (vector stays
            # clean for DVE compute; scalar's triggers all land before the
            # first exp)
            ld_q = [nc.sync, nc.scalar, nc.gpsimd]
            ld_i = [0]

            def ld(kb=128):
                eng = ld_q[ld_i[0] % len(ld_q)]
                ld_i[0] += 1
                return eng

            # x for ALL blocks lives in one resident tile; block-0 columns
            # load as fine 128KB units interleaved with the first wq column-
            # half (consumer order: the opening 4-way chain pair0+K+V^T eats
            # one xt chunk per ~0.9us, matching 3-queue DMA arrival), block 1
            # next, then blocks 2-3 as 256KB units
            xt_all = xtp.tile([P, HO, S], BF16, name="xt_all")
            wq_sb = wpool.tile([P, 2, HO, QC // 2], BF16, name="wq_sb")
            cs_sb = consts.tile([P, NB, 2, SB], BF16, name="cs_sb")
            wkv_sb = wpool.tile([P, HO, 2, HD], BF16, name="wkv_sb")
            c3_sb = consts.tile([P, 3, HD], BF16, name="c3_sb")
            # interleave in consumer order: the opening 4-way chain eats
            # wq-half0[ho], wkv[ho], xt0[ho] per ~0.9us step; first units are
            # small so the first matmul can issue ~1.5us after DMA starts
            # first weight units are tiny (1 ho) so the opening matmul can
            # issue as soon as possible; later units grow to amortize triggers
            wg = [(0, 1), (1, 1), (2, 2), (4, 4), (8, 4), (12, 4)]
            wgi = iter(wg)
            for ho in range(HO):
                if ho in (0, 1, 2, 4, 8, 12):
                    lo, n = next(wgi)
                    ld(64 * n).dma_start(
                        wq_sb[:, 0, lo:lo + n, :], wq_r[:, 0, lo:lo + n, :])
                    ld(64 * n).dma_start(
                        wkv_sb[:, lo:lo + n, :, :], wkv_r[:, lo:lo + n, :, :])
                ld().dma_start(xt_all[:, ho, ts(0, SB)], xt_r[:, ho, ts(0, SB)])
                if ho == 7:
                    ld(256).dma_start(cs_sb[:, 0, :, :], cs_r[:, 0, :, :])
                    ld(96).dma_start(c3_sb[:], consts3[:])
            # all-ones stationary: the rowsum matmul then writes the softmax
            # denominator broadcast across all 128 partitions
            ones_sb = consts.tile([P, HD], BF16, name="ones_sb")
            nc.vector.memset(ones_sb[:], 1.0)
            for hg in range(4):
                ld(256).dma_start(
                    wq_sb[:, 1, 4 * hg:4 * hg + 4, :],
                    wq_r[:, 1, 4 * hg:4 * hg + 4, :])
            for ho in range(HO):
                ld().dma_start(xt_all[:, ho, ts(1, SB)], xt_r[:, ho, ts(1, SB)])
            ld(256).dma_start(cs_sb[:, 1, :, :], cs_r[:, 1, :, :])
            for ho in range(HO):
                ld(256).dma_start(
                    xt_all[:, ho, ds(2 * SB, 2 * SB)], xt_r[:, ho, ds(2 * SB, 2 * SB)])
                if ho == 7:
                    ld(256).dma_start(cs_sb[:, 2, :, :], cs_r[:, 2, :, :])
            ld(256).dma_start(cs_sb[:, 3, :, :], cs_r[:, 3, :, :])
            wo_sb = wpool.tile([P, 4, NC, P], BF16, name="wo_sb")
            for m in range(4):
                ld(512).dma_start(wo_sb[:, m, :, :], wo_r[:, m, :, :])

            # K^T and V for the whole sequence (grow per block)
            kT_sb = kvp.tile([P, S], BF16, name="kT_sb")   # [hd, s]
            v_sb = kvp.tile([P, S], BF16, name="v_sb")     # [s%128, kc*128+hd]

            def rope(out_ap, ps_raw, j):
                """out = raw*cos + (rot @ raw)*sin, written as bf16.

                DVE ops run at high scheduler priority: rope gates the next
                block's attention.
                """
                with tc.high_priority():
                    q_raw = work.tile([P, SB], BF16, tag="qraw", name="q_raw")
                    nc.vector.tensor_copy(q_raw[:], ps_raw[:])
                    ps_rot = psS.tile([P, SB], F32, tag="pss", name="ps_rot")
                    nc.tensor.matmul(
                        ps_rot[:], c3_sb[:, 0, :], q_raw[:], start=True, stop=True)
                    t1 = work.tile([P, SB], BF16, tag="t1", name="t1")
                    nc.vector.tensor_mul(t1[:], q_raw[:], cs_sb[:, j, 0, :])
                    t2 = work.tile([P, SB], BF16, tag="t2", name="t2")
                    nc.vector.tensor_mul(t2[:], ps_rot[:], cs_sb[:, j, 1, :])
                    nc.vector.tensor_add(out_ap, t1[:], t2[:])

            def wq_st(qc, ho):
                return wq_sb[:, qc // 2, ho, ts(qc % 2, P)]

            def qkv_ab(j):
                """Q head-pair 0 + K + V^T as a 4-way chain: all four matmuls
                of a ho-step share the moving xt chunk (and at ramp, one
                arriving chunk feeds ~0.9us of PE work).  K/V^T psums borrow
                the Wo pool, idle during projection phases.  V^T is a single
                accumulation region per bank; interleaving REGIONS of one
                bank corrupts PSUM accumulation groups."""
                q_all = qfp.tile([P, GQ, SB], BF16, name="q_all")
                ps_a = psP.tile([P, SB], F32, tag="pp", name="ps_qa")
                ps_b = psP.tile([P, SB], F32, tag="pp", name="ps_qb")
                ps_k = psW.tile([P, SB], F32, tag="pw", name="ps_k")
                ps_vT = psW.tile([P, SB], F32, tag="pw", name="ps_vT")
                for ho in range(HO):
                    nc.tensor.matmul(
                        ps_a[:], wq_st(0, ho), xt_all[:, ho, ts(j, SB)],
                        start=(ho == 0), stop=(ho == HO - 1))
                    nc.tensor.matmul(
                        ps_b[:], wq_st(1, ho), xt_all[:, ho, ts(j, SB)],
                        start=(ho == 0), stop=(ho == HO - 1))
                    nc.tensor.matmul(
                        ps_k[:], wkv_sb[:, ho, 0, :], xt_all[:, ho, ts(j, SB)],
                        start=(ho == 0), stop=(ho == HO - 1))
                    nc.tensor.matmul(
                        ps_vT[:], wkv_sb[:, ho, 1, :], xt_all[:, ho, ts(j, SB)],
                        start=(ho == 0), stop=(ho == HO - 1))
                rope(kT_sb[:, ts(j, SB)], ps_k, j)
                rope(q_all[:, 0, :], ps_a, j)
                rope(q_all[:, 1, :], ps_b, j)
                # transpose V^T [hd, s] -> V [s%128, hd] chunks via PE
                vT_sb = work.tile([P, SB], BF16, tag="vt", name="vT_sb")
                nc.vector.tensor_copy(vT_sb[:], ps_vT[:])
                ps_vt2 = psP.tile([P, SB], BF16, tag="pp", name="ps_vt2")
                for st in range(ST):
                    nc.tensor.transpose(
                        ps_vt2[:, ts(st, P)], vT_sb[:, ts(st, P)], c3_sb[:, 1, :])
                nc.vector.tensor_copy(v_sb[:, ts(j, SB)], ps_vt2[:])
                return q_all

            def qkv_c(j, q_all):
                ps_a = psP.tile([P, SB], F32, tag="pp", name="ps_qa")
                ps_b = psP.tile([P, SB], F32, tag="pp", name="ps_qb")
                for ho in range(HO):
                    nc.tensor.matmul(
                        ps_a[:], wq_st(2, ho), xt_all[:, ho, ts(j, SB)],
                        start=(ho == 0), stop=(ho == HO - 1))
                    nc.tensor.matmul(
                        ps_b[:], wq_st(3, ho), xt_all[:, ho, ts(j, SB)],
                        start=(ho == 0), stop=(ho == HO - 1))
                rope(q_all[:, 2, :], ps_a, j)
                rope(q_all[:, 3, :], ps_b, j)

            def attn_head(j, q_all, h, o_all):
                """One head's causal attention for query block j.

                Score chunks are emitted in pairs sharing the moving q; the
                exp/acc/PV for the pair follows, so the PE stream is
                s,s,pv,pv,... and ACT latency hides behind the second score.
                """
                KC = 4 * (j + 1)
                ps_o = psO.tile([P, SB], F32, tag="pso", name="ps_o")
                acc = work.tile([P, SB], BF16, tag="acca", name="acc")
                for base in range(0, KC, 2):
                    pair = []
                    for kc in (base, base + 1):
                        if kc >= KC:
                            continue
                        diag = kc >= 4 * j
                        d = P * (kc - 4 * j) if diag else 0
                        ps_s = psS.tile([P, SB], F32, tag="pss", name="ps_s")
                        nc.tensor.matmul(
                            ps_s[:, d:], kT_sb[:, ts(kc, P)], q_all[:, h, d:],
                            start=True, stop=not diag,
                        )
                        if diag:
                            # rank-128 update adds -40 on causally-masked
                            # slots; exp then yields ~0 with no mask op
                            nc.tensor.matmul(
                                ps_s[:, d:d + P], c3_sb[:, 1, :], c3_sb[:, 2, :],
                                start=False, stop=True,
                            )
                        pair.append((kc, d, ps_s))
                    for kc, d, ps_s in pair:
                        pt = ptp.tile([P, SB], BF16, tag="pt", name="pt")
                        nc.scalar.activation(pt[:, d:], ps_s[:, d:], AF.Exp)
                        if kc == 0:
                            nc.vector.tensor_copy(acc[:], pt[:])
                        else:
                            nc.vector.tensor_add(acc[:, d:], acc[:, d:], pt[:, d:])
                        nc.tensor.matmul(
                            ps_o[:, d:], v_sb[:, ts(kc, P)], pt[:, d:],
                            start=(kc == 0), stop=(kc == KC - 1),
                        )
                ps_d = psS.tile([P, SB], F32, tag="pss", name="ps_d")
                nc.tensor.matmul(ps_d[:], ones_sb[:], acc[:], start=True, stop=True)
                rb = work.tile([P, SB], F32, tag="rb", name="rb")
                nc.vector.reciprocal_approx_fast(rb[:], ps_d[:])
                nc.vector.tensor_mul(o_all[:, h, :], ps_o[:], rb[:])

            def attn_phase(j, q_all, o_all, heads):
                for h in heads:
                    attn_head(j, q_all, h, o_all)

            def wo_phase(j, o_all, groups=range(8)):
                """y_partial^T[n-chunks, s-block] += own-head contraction.

                Groups of 2 n-chunks (2 PSUM banks), m-chunks inner; the two
                matmuls of an m-step share the moving O^T chunk.  Evacuation
                alternates DVE/ACT; stores ride the gpsimd queue.
                """
                for g in groups:
                    ps_y0 = psW.tile([P, SB], F32, tag="pw", name="ps_y0")
                    ps_y1 = psW.tile([P, SB], F32, tag="pw", name="ps_y1")
                    for m in range(4):
                        nc.tensor.matmul(
                            ps_y0[:], wo_sb[:, m, 2 * g, :], o_all[:, m, :],
                            start=(m == 0), stop=(m == 3))
                        nc.tensor.matmul(
                            ps_y1[:], wo_sb[:, m, 2 * g + 1, :], o_all[:, m, :],
                            start=(m == 0), stop=(m == 3))
                    for i, ps_y in enumerate((ps_y0, ps_y1)):
                        y_sb = yevac.tile([P, SB], BF16, tag="ysb", name="y_sb")
                        with tc.high_priority():
                            if (g + i) % 2 == 0:
                                nc.vector.tensor_copy(y_sb[:], ps_y[:])
                            else:
                                nc.scalar.activation(y_sb[:], ps_y[:], AF.Copy)
                        st_eng = nc.gpsimd if (g + i) % 2 == 0 else nc.sync
                        st_eng.dma_start(out_r[:, 2 * g + i, j, :], y_sb[:])

            # emission: interleave projections with attention so PE always
            # has dense independent work while ACT chews on exp; attention
            # heads are emitted as soon as their rope is done; each block's
            # Wo partial follows its attention (no cross-core deps anywhere).
            o0, o1, o2, o3 = [
                ofp.tile([P, GQ, SB], BF16, name="o_all") for _ in range(4)]
            q0 = qkv_ab(0)
            attn_phase(0, q0, o0, [0])
            qkv_c(0, q0)
            attn_phase(0, q0, o0, [1])
            attn_phase(0, q0, o0, [2])
            attn_phase(0, q0, o0, [3])
            q1 = qkv_ab(1)
            qkv_c(1, q1)
            attn_phase(1, q1, o1, [0, 1])
            q2 = qkv_ab(2)
            attn_phase(1, q1, o1, [2, 3])
            qkv_c(2, q2)
            q3 = qkv_ab(3)
            attn_phase(2, q2, o2, [0, 1])
            qkv_c(3, q3)
            attn_phase(2, q2, o2, [2, 3])
            wo_phase(0, o0)
            attn_phase(3, q3, o3, [0, 1])
            wo_phase(1, o1)
            attn_phase(3, q3, o3, [2, 3])
            wo_phase(2, o2)
            wo_phase(3, o3)

    return nc


def make_in_maps(x, cos, sin, Wq, Wkv, Wo, S=2048, H=2048):
    bf = ml_dtypes.bfloat16
    scale = 1.0 / math.sqrt(HD)
    NKVH = Wkv.shape[1] // (2 * HD)  # 4
    NB, HO, NC = S // SB, H // P, H // P

    Prot = np.zeros((HD, HD), np.float32)
    Prot[np.arange(64), np.arange(64) + 64] = -1.0
    Prot[np.arange(64) + 64, np.arange(64)] = 1.0
    rotm = np.ascontiguousarray(Prot.T).astype(np.float32)

    kk = np.arange(P)[:, None]
    w = np.arange(HD)[None, :]
    trineg_np = np.where(w < kk, -40.0, 0.0).astype(np.float32)
    ident_np = np.eye(HD, dtype=np.float32)
    consts3 = np.concatenate([rotm, ident_np, trineg_np], axis=1).astype(bf)

    # cossin: [j, p, kind, s]
    cs = np.stack([np.asarray(cos).T, np.asarray(sin).T])           # [2, 128, S]
    cs = cs.reshape(2, P, NB, SB).transpose(2, 1, 0, 3)             # [j, p, k, s]
    cossin = np.ascontiguousarray(cs.reshape(NB * P, 2 * SB)).astype(bf)

    in_maps = []
    for c in range(8):
        b, g = c // 4, c % 4
        # xt: [ho, p, s] -- 4KB per-partition lines
        xtc = np.ascontiguousarray(
            np.asarray(x)[b].T.reshape(HO * P, S)).astype(bf)
        # wkv: [p, {k,v}, ho, m] (one 1MB DMA, 4KB/partition lines)
        wkc = np.asarray(Wkv)[:, HD * g:HD * (g + 1)].reshape(HO, P, HD)
        wvc = np.asarray(Wkv)[:, NKVH * HD + HD * g:NKVH * HD + HD * (g + 1)].reshape(HO, P, HD)
        wkv_c = np.stack([wkc, wvc]).transpose(2, 1, 0, 3)          # [p, ho, k, m]
        wkv_c = np.ascontiguousarray(wkv_c.reshape(P, HO * 2 * HD)).astype(bf)
        # wo: row shard [512, 2048] repacked to [p, m(4), n(16), 128]
        wo_c = np.asarray(Wo)[QC * g:QC * (g + 1), :].reshape(4, P, NC, P)
        wo_c = np.ascontiguousarray(
            wo_c.transpose(1, 0, 2, 3).reshape(P, 4 * NC * P)).astype(bf)
        # wq: [p, half, ho, m] with the rope scale folded in
        wq_c = (np.asarray(Wq)[:, QC * g:QC * (g + 1)] * scale).reshape(
            HO, P, 2, QC // 2)
        wq_c = np.ascontiguousarray(
            wq_c.transpose(1, 2, 0, 3).reshape(P, 2 * HO * (QC // 2))).astype(bf)
        in_maps.append({
            "xt": xtc,
            "wq": wq_c,
            "wkv": wkv_c,
            "wo": wo_c,
            "cossin": cossin, "consts3": consts3,
        })
    return in_maps


_CACHE = {}


def _get_nc(S=2048, H=2048):
    key = (S, H)
    if key not in _CACHE:
        nc = build_kernel(S, H)
        nc.compile()
        _CACHE[key] = nc
    return _CACHE[key]


def run(x, cos, sin, Wq, Wkv, Wo, trace=False):
    S, H = 2048, 2048
    nc = _get_nc(S, H)
    in_maps = make_in_maps(x, cos, sin, Wq, Wkv, Wo, S, H)
    res = bass_utils.run_bass_kernel_spmd(
        nc, in_maps, core_ids=list(range(8)), trace=trace
    )
    # unshard: sum the 4 kv-group partial y^T per batch, transpose back
    NC, NB = H // 128, S // SB
    y = np.empty((2, S, H), np.float32)
    for b in range(2):
        acc = np.zeros((H, S), np.float32)
        for g in range(4):
            part = np.asarray(res.results[4 * b + g]["out"], dtype=np.float32)
            acc += part.reshape(NC, NB, 128, SB).transpose(0, 2, 1, 3).reshape(H, S)
        y[b] = acc.T
    return y, res


def kernel(x, cos, sin, Wq, Wkv, Wo):
    y, _ = run(x, cos, sin, Wq, Wkv, Wo, trace=False)
    return y
